# revision 2
# baseline (speedup 1.0000x reference)
"""Single-head causal attention (B=4, T=4096, C=1024, H=64) on 8 Trainium2 cores.

Sharding: core c = (batch b=c//2, parity p=c%2). Each core handles the 16
query row-blocks (128 rows) with global block index 2t+p, t=0..15 — parity
interleaving balances causal work exactly across the two cores of a batch,
and makes the SPMD program shape-uniform (key extent for local block t is
256*(t+1), independent of parity; the half-block of waste is masked out).

Device per core (fp16 matmul inputs, fp32 PSUM accumulation):
  k^T,v^T = W^T @ x^T (full 4096 keys), q^T for own 2048 rows.
  For each 128-key block j: S^T[128k, q] = kT_j^T q (scores transposed),
  additive diagonal mask (host-provided, parity-dependent), U = exp(S^T/32)
  via ACT, then [out^T; rowsum] += [v_j | 1]^T U accumulated in PSUM.
Host normalizes (out^T / rowsum), transposes, and scatters rows back.
"""
import numpy as np

B, T, C, H = 4, 4096, 1024, 64
TQ = T // 2              # own query rows per core
NKB = T // 128           # 32 key blocks
NEG = -1e9
N_CORES = 8

_cache = {}


def _build_nc():
    import concourse.bass as bass
    import concourse.tile as tile
    from concourse import bacc, mybir
    from concourse.masks import make_identity

    f32 = mybir.dt.float32
    f16 = mybir.dt.float16
    AF = mybir.ActivationFunctionType

    nc = bacc.Bacc()
    xqT = nc.declare_dram_parameter("xqT", [C, TQ], f16, isOutput=False)
    xkT = nc.declare_dram_parameter("xkT", [C, T], f16, isOutput=False)
    wq = nc.declare_dram_parameter("wq", [C, H], f16, isOutput=False)
    wk = nc.declare_dram_parameter("wk", [C, H], f16, isOutput=False)
    wv = nc.declare_dram_parameter("wv", [C, H], f16, isOutput=False)
    mask_e = nc.declare_dram_parameter("mask_e", [128, 128], f32, isOutput=False)
    mask_o = nc.declare_dram_parameter("mask_o", [128, 128], f32, isOutput=False)
    oacc = nc.declare_dram_parameter("oacc", [H + 1, TQ], f32, isOutput=True)

    CA = C // 128  # 8 contraction chunks

    with tile.TileContext(nc) as tc:
        with tc.tile_pool(name="consts", bufs=1) as consts, \
             tc.tile_pool(name="big", bufs=1) as big:
            wq_sb = consts.tile([128, CA, H], f16, tag="wq")
            wk_sb = consts.tile([128, CA, H], f16, tag="wk")
            wv_sb = consts.tile([128, CA, H], f16, tag="wv")
            me_sb = consts.tile([128, 128], f32, tag="me")
            mo_sb = consts.tile([128, 128], f32, tag="mo")
            ident = consts.tile([64, 64], f16, tag="ident")
            for w_sb, w_dr in ((wq_sb, wq), (wk_sb, wk), (wv_sb, wv)):
                nc.sync.dma_start(out=w_sb, in_=w_dr.rearrange("(a p) h -> p a h", p=128))
            nc.sync.dma_start(out=me_sb, in_=mask_e[:, :])
            nc.sync.dma_start(out=mo_sb, in_=mask_o[:, :])
            make_identity(nc, ident)

            kT_sb = big.tile([64, T], f16, tag="kT")
            qT_sb = big.tile([64, TQ], f16, tag="qT")
            v_all = big.tile([128, NKB, H + 1], f16, tag="v")
            nc.vector.memset(v_all[:, :, H:H + 1], 1.0)

            xkT_r = xkT.rearrange("(a p) t -> p a t", p=128)
            xqT_r = xqT.rearrange("(a p) t -> p a t", p=128)

            # ---- Phase B: k^T, v^T projections + v transpose; q^T ----
            with tc.tile_pool(name="xt", bufs=3) as xtp, \
                 tc.tile_pool(name="vt16", bufs=2) as vt16p, \
                 tc.tile_pool(name="pproj", bufs=4, space="PSUM") as pp, \
                 tc.tile_pool(name="pvt", bufs=2, space="PSUM") as pvt:
                for g in range(T // 512):
                    xk_t = xtp.tile([128, CA, 512], f16, tag="xt")
                    nc.sync.dma_start(out=xk_t, in_=xkT_r[:, :, 512 * g:512 * (g + 1)])
                    ps_k = pp.tile([64, 512], f32, tag="pp")
                    ps_v = pp.tile([64, 512], f32, tag="pp")
                    for a in range(CA):
                        nc.tensor.matmul(ps_k, lhsT=wk_sb[:, a, :], rhs=xk_t[:, a, :],
                                         start=(a == 0), stop=(a == CA - 1))
                    for a in range(CA):
                        nc.tensor.matmul(ps_v, lhsT=wv_sb[:, a, :], rhs=xk_t[:, a, :],
                                         start=(a == 0), stop=(a == CA - 1))
                    nc.scalar.activation(kT_sb[:, 512 * g:512 * (g + 1)], ps_k, AF.Copy)
                    vt16 = vt16p.tile([64, 512], f16, tag="vt16")
                    nc.scalar.activation(vt16, ps_v, AF.Copy)
                    for u in range(4):
                        j = 4 * g + u
                        ps_t = pvt.tile([128, 64], f16, tag="pvt")
                        nc.tensor.transpose(ps_t, vt16[:, 128 * u:128 * (u + 1)], ident)
                        nc.vector.tensor_copy(v_all[:, j, 0:H], ps_t)
                for g in range(TQ // 512):
                    xq_t = xtp.tile([128, CA, 512], f16, tag="xt")
                    nc.sync.dma_start(out=xq_t, in_=xqT_r[:, :, 512 * g:512 * (g + 1)])
                    ps_q = pp.tile([64, 512], f32, tag="pp")
                    for a in range(CA):
                        nc.tensor.matmul(ps_q, lhsT=wq_sb[:, a, :], rhs=xq_t[:, a, :],
                                         start=(a == 0), stop=(a == CA - 1))
                    nc.scalar.activation(qT_sb[:, 512 * g:512 * (g + 1)], ps_q, AF.Copy)

            # ---- Phase C: scores^T -> mask -> exp -> AV accumulate ----
            with tc.tile_pool(name="pss", bufs=2, space="PSUM") as pss, \
                 tc.tile_pool(name="po", bufs=1, space="PSUM") as pop, \
                 tc.tile_pool(name="u16", bufs=3) as up, \
                 tc.tile_pool(name="obuf", bufs=2) as obp:
                ps_o = pop.tile([H + 1, TQ], f32, tag="po")
                for j in range(NKB):
                    t0 = j // 2
                    q_start = 128 * t0
                    ss = q_start
                    first_slab = True
                    while ss < TQ:
                        se = min((ss // 1024 + 1) * 1024, TQ)
                        w = se - ss
                        ps_s = pss.tile([128, 1024], f32, tag="pss")
                        ls = 0
                        while ls < w:  # scores pieces: ps_s-local 512 grid
                            le = min(ls + 512, w)
                            nc.tensor.matmul(
                                ps_s[:, ls:le], lhsT=kT_sb[:, 128 * j:128 * (j + 1)],
                                rhs=qT_sb[:, ss + ls:ss + le], start=True, stop=True)
                            ls = le
                        if first_slab:
                            m = me_sb if j % 2 == 0 else mo_sb
                            nc.vector.tensor_add(ps_s[:, 0:128], ps_s[:, 0:128], m)
                        u_sb = up.tile([128, 1024], f16, tag="u16")
                        nc.scalar.activation(u_sb[:, 0:w], ps_s[:, 0:w], AF.Exp,
                                             scale=float(C ** -0.5))
                        gs = ss
                        while gs < se:  # AV pieces: ps_o-global 512 grid
                            ge = min((gs // 512 + 1) * 512, se)
                            nc.tensor.matmul(
                                ps_o[:, gs:ge], lhsT=v_all[:, j, :],
                                rhs=u_sb[:, gs - ss:ge - ss], start=(j == 0),
                                stop=(j == NKB - 1), skip_group_check=True)
                            gs = ge
                        first_slab = False
                        ss = se
                    if j % 8 == 7:  # bank g of ps_o is final after j == 8g+7
                        g = (j - 7) // 8
                        osb_g = obp.tile([H + 1, 512], f32, tag="osb")
                        nc.vector.tensor_copy(osb_g, ps_o[:, 512 * g:512 * (g + 1)])
                        nc.sync.dma_start(out=oacc[:, 512 * g:512 * (g + 1)], in_=osb_g)
    nc.compile()
    return nc


def _get_nc():
    if "nc" not in _cache:
        _cache["nc"] = _build_nc()
    return _cache["nc"]


def _core_masks(p):
    kk = np.arange(128)[:, None]
    i = np.arange(128)[None, :]
    tri = np.where(i >= kk, 0.0, NEG).astype(np.float32)
    if p == 0:
        return tri, np.full((128, 128), NEG, np.float32)
    return np.zeros((128, 128), np.float32), tri


def make_in_maps(x, Wk, Wq, Wv):
    wk16 = np.ascontiguousarray(Wk).astype(np.float16)
    wq16 = np.ascontiguousarray(Wq).astype(np.float16)
    wv16 = np.ascontiguousarray(Wv).astype(np.float16)
    in_maps = []
    for c in range(N_CORES):
        b, p = c // 2, c % 2
        xb = np.asarray(x[b])
        xq = xb.reshape(NKB, 128, C)[p::2].reshape(TQ, C)
        xqT = np.ascontiguousarray(xq.T.astype(np.float16))
        xkT = np.ascontiguousarray(xb.T.astype(np.float16))
        me, mo = _core_masks(p)
        in_maps.append({"xqT": xqT, "xkT": xkT, "wq": wq16, "wk": wk16,
                        "wv": wv16, "mask_e": me, "mask_o": mo})
    return in_maps


def postprocess(results):
    out = np.zeros((B, T, H), np.float32)
    for c in range(N_CORES):
        b, p = c // 2, c % 2
        acc = results[c]["oacc"]
        o = (acc[0:H] / acc[H:H + 1]).T
        out[b].reshape(NKB, 128, H)[p::2] = o.reshape(16, 128, H)
    return out


def run_full(x, Wk, Wq, Wv, trace=False):
    from concourse.bass_utils import run_bass_kernel_spmd
    nc = _get_nc()
    in_maps = make_in_maps(x, Wk, Wq, Wv)
    res = run_bass_kernel_spmd(nc, in_maps, list(range(N_CORES)), trace=trace)
    return postprocess(res.results), res


def kernel(x, Wk, Wq, Wv):
    out, _ = run_full(x, Wk, Wq, Wv)
    return out


# revision 4
# speedup vs baseline: 1.2576x; 1.2576x over previous
"""Single-head causal attention (B=4, T=4096, C=1024, H=64) on 8 Trainium2 cores.

Sharding: core c = (batch b=c//2, parity p=c%2). Each core handles the 16
query row-blocks (128 rows) with global block index 2t+p, t=0..15 — parity
interleaving balances causal work exactly across the two cores of a batch
and keeps the SPMD program shape-uniform (key extent for local block t is
256*(t+1) regardless of parity; the half-block of waste is masked out).

Device per core (fp16 matmul inputs, fp32 PSUM accumulation):
  k^T,v^T = W^T x^T (full 4096 keys), q^T for own 2048 rows.  Attention is
  computed in scores-transposed layout: for key block j, S^T[128k, q] =
  kT_j^T q, additive diagonal mask (host-provided, parity-dependent),
  U = exp(S^T/32) on ACT, then [out^T; rowsum] += [v_j | 1]^T U in PSUM.
Projections and attention are interleaved per 512-key group so the PE
stream stays dense (HAM stays warm), and the q range is covered in two
1024-column passes so PSUM fits (scores 2 slabs x 2 banks + out 2 banks +
projection 2 banks = 8).  Host normalizes (out^T / rowsum) and scatters.
"""
import numpy as np

B, T, C, H = 4, 4096, 1024, 64
TQ = T // 2              # own query rows per core
NKB = T // 128           # 32 key blocks
NG = T // 512            # 8 key projection groups
NEG = -1e9
N_CORES = 8

_cache = {}


def _build_nc():
    import concourse.bass as bass
    import concourse.tile as tile
    from concourse import bacc, mybir
    from concourse.masks import make_identity

    f32 = mybir.dt.float32
    f16 = mybir.dt.float16
    AF = mybir.ActivationFunctionType

    nc = bacc.Bacc()
    xq_p = nc.declare_dram_parameter("xq_p", [4, 128, 8, 512], f16, isOutput=False)
    xk_p = nc.declare_dram_parameter("xk_p", [8, 128, 8, 512], f16, isOutput=False)
    wq = nc.declare_dram_parameter("wq", [C, H], f16, isOutput=False)
    wk = nc.declare_dram_parameter("wk", [C, H], f16, isOutput=False)
    wv = nc.declare_dram_parameter("wv", [C, H], f16, isOutput=False)
    mask_e = nc.declare_dram_parameter("mask_e", [128, 128], f32, isOutput=False)
    mask_o = nc.declare_dram_parameter("mask_o", [128, 128], f32, isOutput=False)
    oacc = nc.declare_dram_parameter("oacc", [H + 1, TQ], f32, isOutput=True)

    CA = C // 128  # 8 contraction chunks

    with tile.TileContext(nc) as tc:
        with tc.tile_pool(name="consts", bufs=1) as consts, \
             tc.tile_pool(name="big", bufs=1) as big, \
             tc.tile_pool(name="xt", bufs=3) as xtp, \
             tc.tile_pool(name="vt16", bufs=2) as vt16p, \
             tc.tile_pool(name="u16", bufs=3) as up, \
             tc.tile_pool(name="obuf", bufs=2) as obp, \
             tc.tile_pool(name="pproj", bufs=2, space="PSUM") as pp, \
             tc.tile_pool(name="pss", bufs=2, space="PSUM") as pss, \
             tc.tile_pool(name="po", bufs=1, space="PSUM") as pop:

            wq_sb = consts.tile([128, CA, H], f16, tag="wq")
            wk_sb = consts.tile([128, CA, H], f16, tag="wk")
            wv_sb = consts.tile([128, CA, H], f16, tag="wv")
            me_sb = consts.tile([128, 128], f32, tag="me")
            mo_sb = consts.tile([128, 128], f32, tag="mo")
            ident = consts.tile([64, 64], f16, tag="ident")
            for w_sb, w_dr in ((wq_sb, wq), (wk_sb, wk), (wv_sb, wv)):
                nc.sync.dma_start(out=w_sb, in_=w_dr.rearrange("(a p) h -> p a h", p=128))
            nc.sync.dma_start(out=me_sb, in_=mask_e[:, :])
            nc.sync.dma_start(out=mo_sb, in_=mask_o[:, :])
            make_identity(nc, ident)

            kT_sb = big.tile([64, T], f16, tag="kT")
            qT_sb = big.tile([64, TQ], f16, tag="qT")
            v_all = big.tile([128, NKB, H + 1], f16, tag="v")
            nc.vector.memset(v_all[:, :, H:H + 1], 1.0)

            def proj_group(w_sb, dram_g, out_sb):
                """One 512-col projection group: out_sb[64,512] f16 via DVE."""
                x_t = xtp.tile([128, CA, 512], f16, tag="xt")
                nc.sync.dma_start(out=x_t, in_=dram_g)
                ps = pp.tile([64, 512], f32, tag="pp")
                for a in range(CA):
                    nc.tensor.matmul(ps, lhsT=w_sb[:, a, :], rhs=x_t[:, a, :],
                                     start=(a == 0), stop=(a == CA - 1))
                nc.vector.tensor_copy(out_sb, ps)

            def attn_j(j, lo, hi, ps_o, po_base):
                """Attention for key block j over q columns [max(q_start,lo), hi)."""
                t0 = j // 2
                q_start = 128 * t0
                ss = max(q_start, lo)
                if ss >= hi:
                    return
                w = hi - ss
                ps_s = pss.tile([128, 1024], f32, tag="pss")
                ls = 0
                while ls < w:  # scores pieces: ps_s-local 512 grid
                    le = min(ls + 512, w)
                    nc.tensor.matmul(
                        ps_s[:, ls:le], lhsT=kT_sb[:, 128 * j:128 * (j + 1)],
                        rhs=qT_sb[:, ss + ls:ss + le], start=True, stop=True)
                    ls = le
                if ss == q_start:  # diagonal in this pass -> mask
                    m = me_sb if j % 2 == 0 else mo_sb
                    nc.vector.tensor_add(ps_s[:, 0:128], ps_s[:, 0:128], m)
                u_sb = up.tile([128, 1024], f16, tag="u16")
                nc.scalar.activation(u_sb[:, 0:w], ps_s[:, 0:w], AF.Exp,
                                     scale=float(C ** -0.5))
                gs = ss
                while gs < hi:  # AV pieces: global 512 grid
                    ge = min((gs // 512 + 1) * 512, hi)
                    nc.tensor.matmul(
                        ps_o[:, gs - po_base:ge - po_base], lhsT=v_all[:, j, :],
                        rhs=u_sb[:, gs - ss:ge - ss], start=(j == 0),
                        stop=False, skip_group_check=True)
                    gs = ge

            def kv_group(g, attn_js, attn_fn):
                """Project keys/values for group g; interleave attention calls."""
                x_t = xtp.tile([128, CA, 512], f16, tag="xt")
                nc.sync.dma_start(out=x_t, in_=xk_p[g])
                ps_k = pp.tile([64, 512], f32, tag="pp")
                for a in range(CA):
                    nc.tensor.matmul(ps_k, lhsT=wk_sb[:, a, :], rhs=x_t[:, a, :],
                                     start=(a == 0), stop=(a == CA - 1))
                nc.vector.tensor_copy(kT_sb[:, 512 * g:512 * (g + 1)], ps_k)
                ps_v = pp.tile([64, 512], f32, tag="pp")
                for a in range(CA):
                    nc.tensor.matmul(ps_v, lhsT=wv_sb[:, a, :], rhs=x_t[:, a, :],
                                     start=(a == 0), stop=(a == CA - 1))
                vt16 = vt16p.tile([64, 512], f16, tag="vt16")
                nc.vector.tensor_copy(vt16, ps_v)
                # transposes of this group's v blocks + interleaved attention
                for i, j in enumerate(range(4 * g, 4 * g + 4)):
                    ps_t = pss.tile([128, 64], f16, tag="pss")
                    nc.tensor.transpose(ps_t, vt16[:, 128 * i:128 * (i + 1)], ident)
                    nc.vector.tensor_copy(v_all[:, j, 0:H], ps_t)
                    if i < len(attn_js):
                        attn_fn(attn_js[i])
                for j in attn_js[4:]:
                    attn_fn(j)

            def flush(ps_o, po_base, cols):
                for (c0, c1) in cols:
                    ob = obp.tile([H + 1, 512], f32, tag="ob")
                    nc.vector.tensor_copy(ob[:, 0:c1 - c0], ps_o[:, c0 - po_base:c1 - po_base])
                    nc.sync.dma_start(out=oacc[:, c0:c1], in_=ob[:, 0:c1 - c0])

            # ---- q^T projections ----
            for g in range(TQ // 512):
                proj_group(wq_sb, xq_p[g], qT_sb[:, 512 * g:512 * (g + 1)])

            # ---- pass 0: q cols [0, 1024), key blocks 0..15, kv groups 0..3 ----
            ps_o0 = pop.tile([H + 1, 1024], f32, tag="po")
            for g in range(4):
                js = list(range(4 * g, 4 * g + 4))
                kv_group(g, js, lambda j: attn_j(j, 0, 1024, ps_o0, 0))
            flush(ps_o0, 0, [(0, 512), (512, 1024)])

            # ---- pass 1: q cols [1024, 2048), key blocks 0..31, kv groups 4..7 ----
            ps_o1 = pop.tile([H + 1, 1024], f32, tag="po")
            for g in range(4, NG):
                js = list(range(4 * (g - 4), 4 * (g - 4) + 4))
                kv_group(g, js, lambda j: attn_j(j, 1024, 2048, ps_o1, 1024))
            for j in range(16, NKB):
                attn_j(j, 1024, 2048, ps_o1, 1024)
                if j == 23:
                    flush(ps_o1, 1024, [(1024, 1536)])
            flush(ps_o1, 1024, [(1536, 2048)])
    nc.compile()
    return nc


def _get_nc():
    if "nc" not in _cache:
        _cache["nc"] = _build_nc()
    return _cache["nc"]


def _core_masks(p):
    kk = np.arange(128)[:, None]
    i = np.arange(128)[None, :]
    tri = np.where(i >= kk, 0.0, NEG).astype(np.float32)
    if p == 0:
        return tri, np.full((128, 128), NEG, np.float32)
    return np.zeros((128, 128), np.float32), tri


def _pack(xT16):
    """[1024, W] f16 -> [W//512, 128, 8, 512] contiguous DMA tiles."""
    W = xT16.shape[1]
    return np.ascontiguousarray(
        xT16.reshape(8, 128, W // 512, 512).transpose(2, 1, 0, 3))


def make_in_maps(x, Wk, Wq, Wv):
    wk16 = np.ascontiguousarray(Wk).astype(np.float16)
    wq16 = np.ascontiguousarray(Wq).astype(np.float16)
    wv16 = np.ascontiguousarray(Wv).astype(np.float16)
    in_maps = []
    for c in range(N_CORES):
        b, p = c // 2, c % 2
        xb = np.asarray(x[b])
        xq = xb.reshape(NKB, 128, C)[p::2].reshape(TQ, C)
        xq_p = _pack(xq.T.astype(np.float16))
        xk_p = _pack(xb.T.astype(np.float16))
        me, mo = _core_masks(p)
        in_maps.append({"xq_p": xq_p, "xk_p": xk_p, "wq": wq16, "wk": wk16,
                        "wv": wv16, "mask_e": me, "mask_o": mo})
    return in_maps


def postprocess(results):
    out = np.zeros((B, T, H), np.float32)
    for c in range(N_CORES):
        b, p = c // 2, c % 2
        acc = results[c]["oacc"]
        o = (acc[0:H] / acc[H:H + 1]).T
        out[b].reshape(NKB, 128, H)[p::2] = o.reshape(16, 128, H)
    return out


def run_full(x, Wk, Wq, Wv, trace=False):
    from concourse.bass_utils import run_bass_kernel_spmd
    nc = _get_nc()
    in_maps = make_in_maps(x, Wk, Wq, Wv)
    res = run_bass_kernel_spmd(nc, in_maps, list(range(N_CORES)), trace=trace)
    return postprocess(res.results), res


def kernel(x, Wk, Wq, Wv):
    out, _ = run_full(x, Wk, Wq, Wv)
    return out


# revision 7
# speedup vs baseline: 1.3085x; 1.0405x over previous
"""Single-head causal attention (B=4, T=4096, C=1024, H=64) on 8 Trainium2 cores.

Sharding: core c = (batch b=c//2, parity p=c%2). Each core handles the 16
query row-blocks (128 rows) with global block index 2t+p, t=0..15 — parity
interleaving balances causal work exactly across the two cores of a batch
and keeps the SPMD program shape-uniform (key extent for local block t is
256*(t+1) regardless of parity; the half-block of waste is masked out).

Device per core (fp16 matmul inputs, fp32 PSUM accumulation):
  k^T,v^T = W^T x^T (full 4096 keys), q^T for own 2048 rows.  Attention is
  computed in scores-transposed layout: for key block j, S^T[128k, q] =
  kT_j^T q, additive diagonal mask (host-provided, parity-dependent),
  U = exp(S^T/32) on ACT, then [out^T; rowsum] += [v_j | 1]^T U in PSUM.
Projections and attention are interleaved per 512-key group so the PE
stream stays dense (HAM stays warm), and the q range is covered in two
1024-column passes so PSUM fits (scores 2 slabs x 2 banks + out 2 banks +
projection 2 banks = 8).  Host normalizes (out^T / rowsum) and scatters.
"""
import numpy as np

B, T, C, H = 4, 4096, 1024, 64
TQ = T // 2              # own query rows per core
NKB = T // 128           # 32 key blocks
NG = T // 512            # 8 key projection groups
NEG = -1e9
N_CORES = 8

_cache = {}


def _build_nc():
    import concourse.bass as bass
    import concourse.tile as tile
    from concourse import bacc, mybir
    from concourse.masks import make_identity

    f32 = mybir.dt.float32
    f16 = mybir.dt.float16
    AF = mybir.ActivationFunctionType

    nc = bacc.Bacc()
    xq_p = nc.declare_dram_parameter("xq_p", [4, 128, 8, 512], f16, isOutput=False)
    xk_p = nc.declare_dram_parameter("xk_p", [8, 128, 8, 512], f16, isOutput=False)
    wq = nc.declare_dram_parameter("wq", [C, H], f16, isOutput=False)
    wk = nc.declare_dram_parameter("wk", [C, H], f16, isOutput=False)
    wv = nc.declare_dram_parameter("wv", [C, H], f16, isOutput=False)
    mask_e = nc.declare_dram_parameter("mask_e", [128, 128], f32, isOutput=False)
    mask_o = nc.declare_dram_parameter("mask_o", [128, 128], f32, isOutput=False)
    oacc = nc.declare_dram_parameter("oacc", [H + 1, TQ], f32, isOutput=True)

    CA = C // 128  # 8 contraction chunks

    with tile.TileContext(nc) as tc:
        with tc.tile_pool(name="consts", bufs=1) as consts, \
             tc.tile_pool(name="big", bufs=1) as big, \
             tc.tile_pool(name="xt", bufs=3) as xtp, \
             tc.tile_pool(name="vt16", bufs=2) as vt16p, \
             tc.tile_pool(name="u16", bufs=3) as up, \
             tc.tile_pool(name="obuf", bufs=2) as obp, \
             tc.tile_pool(name="pproj", bufs=2, space="PSUM") as pp, \
             tc.tile_pool(name="pss", bufs=2, space="PSUM") as pss, \
             tc.tile_pool(name="po", bufs=1, space="PSUM") as pop:

            wq_sb = consts.tile([128, CA, H], f16, tag="wq")
            wk_sb = consts.tile([128, CA, H], f16, tag="wk")
            wv_sb = consts.tile([128, CA, H], f16, tag="wv")
            me_sb = consts.tile([128, 128], f32, tag="me")
            mo_sb = consts.tile([128, 128], f32, tag="mo")
            ident = consts.tile([64, 64], f16, tag="ident")
            # wq on the HWDGE path first (first matmul's critical path);
            # the rest via SWDGE so their issue overlaps the xq transfers.
            nc.sync.dma_start(out=wq_sb, in_=wq.rearrange("(a p) h -> p a h", p=128))
            for w_sb, w_dr in ((wk_sb, wk), (wv_sb, wv)):
                nc.gpsimd.dma_start(out=w_sb, in_=w_dr.rearrange("(a p) h -> p a h", p=128))
            nc.gpsimd.dma_start(out=me_sb, in_=mask_e[:, :])
            nc.gpsimd.dma_start(out=mo_sb, in_=mask_o[:, :])
            make_identity(nc, ident)

            kT_sb = big.tile([64, T], f16, tag="kT")
            qT_sb = big.tile([64, TQ], f16, tag="qT")
            v_all = big.tile([128, NKB, H + 1], f16, tag="v")
            nc.vector.memset(v_all[:, :, H:H + 1], 1.0)

            def proj_group(w_sb, dram_g, out_sb):
                """One 512-col projection group: out_sb[64,512] f16 via DVE."""
                x_t = xtp.tile([128, CA, 512], f16, tag="xt")
                nc.sync.dma_start(out=x_t, in_=dram_g)
                ps = pp.tile([64, 512], f32, tag="pp")
                for a in range(CA):
                    nc.tensor.matmul(ps, lhsT=w_sb[:, a, :], rhs=x_t[:, a, :],
                                     start=(a == 0), stop=(a == CA - 1))
                nc.vector.tensor_copy(out_sb, ps)

            def attn_j(j, lo, hi, ps_o, po_base):
                """Attention for key block j over q columns [max(q_start,lo), hi)."""
                t0 = j // 2
                q_start = 128 * t0
                ss = max(q_start, lo)
                if ss >= hi:
                    return
                w = hi - ss
                ps_s = pss.tile([128, 1024], f32, tag="pss")
                ls = 0
                while ls < w:  # scores pieces: ps_s-local 512 grid
                    le = min(ls + 512, w)
                    nc.tensor.matmul(
                        ps_s[:, ls:le], lhsT=kT_sb[:, 128 * j:128 * (j + 1)],
                        rhs=qT_sb[:, ss + ls:ss + le], start=True, stop=True)
                    ls = le
                if ss == q_start:  # diagonal in this pass -> mask
                    m = me_sb if j % 2 == 0 else mo_sb
                    nc.vector.tensor_add(ps_s[:, 0:128], ps_s[:, 0:128], m)
                u_sb = up.tile([128, 1024], f16, tag="u16")
                nc.scalar.activation(u_sb[:, 0:w], ps_s[:, 0:w], AF.Exp,
                                     scale=float(C ** -0.5))
                gs = ss
                while gs < hi:  # AV pieces: global 512 grid
                    ge = min((gs // 512 + 1) * 512, hi)
                    nc.tensor.matmul(
                        ps_o[:, gs - po_base:ge - po_base], lhsT=v_all[:, j, :],
                        rhs=u_sb[:, gs - ss:ge - ss], start=(j == 0),
                        stop=False, skip_group_check=True)
                    gs = ge

            def kv_group(g, attn_js, attn_fn):
                """Project keys/values for group g; interleave attention calls."""
                x_t = xtp.tile([128, CA, 512], f16, tag="xt")
                nc.sync.dma_start(out=x_t, in_=xk_p[g])
                ps_k = pp.tile([64, 512], f32, tag="pp")
                for a in range(CA):
                    nc.tensor.matmul(ps_k, lhsT=wk_sb[:, a, :], rhs=x_t[:, a, :],
                                     start=(a == 0), stop=(a == CA - 1))
                nc.vector.tensor_copy(kT_sb[:, 512 * g:512 * (g + 1)], ps_k)
                ps_v = pp.tile([64, 512], f32, tag="pp")
                for a in range(CA):
                    nc.tensor.matmul(ps_v, lhsT=wv_sb[:, a, :], rhs=x_t[:, a, :],
                                     start=(a == 0), stop=(a == CA - 1))
                vt16 = vt16p.tile([64, 512], f16, tag="vt16")
                nc.vector.tensor_copy(vt16, ps_v)
                # transposes of this group's v blocks + interleaved attention
                for i, j in enumerate(range(4 * g, 4 * g + 4)):
                    ps_t = pss.tile([128, 64], f16, tag="pss")
                    nc.tensor.transpose(ps_t, vt16[:, 128 * i:128 * (i + 1)], ident)
                    nc.vector.tensor_copy(v_all[:, j, 0:H], ps_t)
                    if i < len(attn_js):
                        attn_fn(attn_js[i])
                for j in attn_js[4:]:
                    attn_fn(j)

            def flush(ps_o, po_base, cols):
                for (c0, c1) in cols:
                    ob = obp.tile([H + 1, 512], f32, tag="ob")
                    nc.vector.tensor_copy(ob[:, 0:c1 - c0], ps_o[:, c0 - po_base:c1 - po_base])
                    nc.sync.dma_start(out=oacc[:, c0:c1], in_=ob[:, 0:c1 - c0])

            # ---- q^T projections for pass 0 (cols [0,1024) need groups 0,1) ----
            proj_group(wq_sb, xq_p[0], qT_sb[:, 0:512])
            proj_group(wq_sb, xq_p[1], qT_sb[:, 512:1024])

            # ---- pass 0: q cols [0, 1024), key blocks 0..15, kv groups 0..3 ----
            ps_o0 = pop.tile([H + 1, 1024], f32, tag="po")
            for g in range(4):
                js = list(range(4 * g, 4 * g + 4))
                kv_group(g, js, lambda j: attn_j(j, 0, 1024, ps_o0, 0))
                if g < 2:  # remaining q^T groups, interleaved
                    proj_group(wq_sb, xq_p[g + 2],
                               qT_sb[:, 512 * (g + 2):512 * (g + 3)])
            flush(ps_o0, 0, [(0, 512), (512, 1024)])

            # ---- pass 1: q cols [1024, 2048), key blocks 0..31, kv groups 4..7.
            # Old blocks (0..15, kv ready) interleave with this group's new
            # blocks (4g..4g+3) so the PE stream stays dense to the end. ----
            ps_o1 = pop.tile([H + 1, 1024], f32, tag="po")
            olds = {4: list(range(0, 6)), 5: list(range(6, 12)),
                    6: list(range(12, 16)), 7: []}
            for g in range(4, NG):
                old = olds[g]
                new = list(range(4 * g, 4 * g + 4))
                inter = [j for pair in zip(old, new) for j in pair]
                used = set(inter)
                inter += [j for j in old + new if j not in used]
                if g == 7:  # olds 0..15 and news 16..23 all done by now
                    flush(ps_o1, 1024, [(1024, 1536)])
                kv_group(g, inter, lambda j: attn_j(j, 1024, 2048, ps_o1, 1024))
            flush(ps_o1, 1024, [(1536, 2048)])
    nc.compile()
    return nc


def _get_nc():
    if "nc" not in _cache:
        _cache["nc"] = _build_nc()
    return _cache["nc"]


def _core_masks(p):
    kk = np.arange(128)[:, None]
    i = np.arange(128)[None, :]
    tri = np.where(i >= kk, 0.0, NEG).astype(np.float32)
    if p == 0:
        return tri, np.full((128, 128), NEG, np.float32)
    return np.zeros((128, 128), np.float32), tri


def _pack(xT16):
    """[1024, W] f16 -> [W//512, 128, 8, 512] contiguous DMA tiles."""
    W = xT16.shape[1]
    return np.ascontiguousarray(
        xT16.reshape(8, 128, W // 512, 512).transpose(2, 1, 0, 3))


def make_in_maps(x, Wk, Wq, Wv):
    wk16 = np.ascontiguousarray(Wk).astype(np.float16)
    wq16 = np.ascontiguousarray(Wq).astype(np.float16)
    wv16 = np.ascontiguousarray(Wv).astype(np.float16)
    in_maps = []
    for c in range(N_CORES):
        b, p = c // 2, c % 2
        xb = np.asarray(x[b])
        xq = xb.reshape(NKB, 128, C)[p::2].reshape(TQ, C)
        xq_p = _pack(xq.T.astype(np.float16))
        xk_p = _pack(xb.T.astype(np.float16))
        me, mo = _core_masks(p)
        in_maps.append({"xq_p": xq_p, "xk_p": xk_p, "wq": wq16, "wk": wk16,
                        "wv": wv16, "mask_e": me, "mask_o": mo})
    return in_maps


def postprocess(results):
    out = np.zeros((B, T, H), np.float32)
    for c in range(N_CORES):
        b, p = c // 2, c % 2
        acc = results[c]["oacc"]
        o = (acc[0:H] / acc[H:H + 1]).T
        out[b].reshape(NKB, 128, H)[p::2] = o.reshape(16, 128, H)
    return out


def run_full(x, Wk, Wq, Wv, trace=False):
    from concourse.bass_utils import run_bass_kernel_spmd
    nc = _get_nc()
    in_maps = make_in_maps(x, Wk, Wq, Wv)
    res = run_bass_kernel_spmd(nc, in_maps, list(range(N_CORES)), trace=trace)
    return postprocess(res.results), res


def kernel(x, Wk, Wq, Wv):
    out, _ = run_full(x, Wk, Wq, Wv)
    return out
